# revision 22
# baseline (speedup 1.0000x reference)
"""Trainium2 Bass kernel for triangle (AlphaFold-style) gated attention over pair rows.

Problem: B=1, N=256 rows; per row n: attention over 256 positions,
H=4 heads x CH=32, C=128 channels, additive mask bias (per row, per key),
triangle bias (per head, q, k; shared across rows), sigmoid gating,
output projection. Rows sharded across 8 NeuronCores (32 rows/core), SPMD.

Per-core dataflow (transposed so the softmax key-reduction lands on the
PE partition axis; all matmul operands fp16 = single-pass PE):
  - load X natural (fp32->fp16 cast DMA on gpsimd), PE-transpose to
    xqT/xkT [c=128, tok=256] fp16
  - scores via host-precomputed M_h = wk_h.T wq_h * scale * 256 (fp16;
    the x256 keeps M out of fp16 subnormals; exp compensates with
    scale=1/256): u_h = M_h.T @ xkT, sT_h = u_slice.T @ xqT (all K=128,
    base partition 0 - row-tiled K=32 matmuls crash this HW)
  - triangle bias (x256, fp16) added by identity-matmul accumulation
  - p = exp((sT+tri)/256 + mask) via one ACT op per k-tile [128,1024],
    mask is the per-partition bias; no max-subtraction needed
    (|s|+|tri| bounded, exp(-1e9)=0 exactly like the reference mask)
  - oT[hd,q] = sum_kt v_h.T @ p_h (col-tiled, fp16); denominators
    broadcast directly to [128,256] by block-expander matmuls (2.0 in
    head blocks; the 2.0 folds the tanh-form sigmoid's 0.5), then one
    reciprocal_approx_fast (~18 bits)
  - gating via tanh (same ACT table set as exp; sigmoid would force a
    ~2.7us table reload per row): g = 1 + tanh(lin/2 + bg/2)
  - out[q,c] = ((oT * g * rb) @ wo.T + 1 x bo) natural layout
"""
import numpy as np

B, N, CQ, H, CH = 1, 256, 128, 4, 32
NCORES = 8
ROWS = N // NCORES  # 32
HD = H * CH  # 128


def build_program(rows):
    import concourse.bass as bass
    import concourse.bacc as bacc
    import concourse.mybir as mybir
    from concourse import tile

    f32 = mybir.dt.float32
    fp16 = mybir.dt.float16
    AF = mybir.ActivationFunctionType
    nc = bacc.Bacc("TRN2", target_bir_lowering=False, debug=False)

    qx = nc.declare_dram_parameter("qx", [rows, N, CQ], f32, isOutput=False)
    kvx = nc.declare_dram_parameter("kvx", [rows, N, CQ], f32, isOutput=False)
    maskc = nc.declare_dram_parameter("maskc", [rows, 128, 2], f32, isOutput=False)
    triT = nc.declare_dram_parameter("triT", [2 * H, 128, N], fp16, isOutput=False)
    mcat = nc.declare_dram_parameter("mcat", [CQ, H * CQ], fp16, isOutput=False)
    wvT = nc.declare_dram_parameter("wvT", [CQ, HD], fp16, isOutput=False)
    wgT = nc.declare_dram_parameter("wgT", [CQ, HD], fp16, isOutput=False)
    woT = nc.declare_dram_parameter("woT", [HD, CQ], fp16, isOutput=False)
    bgc = nc.declare_dram_parameter("bgc", [HD, 1], f32, isOutput=False)
    bor = nc.declare_dram_parameter("bor", [1, CQ], fp16, isOutput=False)
    onesr = nc.declare_dram_parameter("onesr", [1, 128], fp16, isOutput=False)
    sele = nc.declare_dram_parameter("sele", [128, H * 128], fp16, isOutput=False)
    id16 = nc.declare_dram_parameter("id16", [128, 128], fp16, isOutput=False)
    out = nc.declare_dram_parameter("out", [rows, N, CQ], f32, isOutput=True)

    with tile.TileContext(nc) as tc:
        with (
            nc.allow_low_precision(reason="fp16 matmul operands and "
                                   "reciprocal_approx_fast by design"),
            tc.tile_pool(name="const", bufs=1) as cp,
            tc.tile_pool(name="sb", bufs=2) as sb,
            tc.tile_pool(name="sbp", bufs=3) as sbp,
            tc.tile_pool(name="ps", bufs=1, space=bass.MemorySpace.PSUM) as ps,
            tc.tile_pool(name="ps2", bufs=2, space=bass.MemorySpace.PSUM) as ps2,
        ):
            # ---- constants ----
            m_s = cp.tile([CQ, H * CQ], fp16, tag="mcat")
            wv_s = cp.tile([CQ, HD], fp16, tag="wv")
            wg_s = cp.tile([CQ, HD], fp16, tag="wg")
            wo_s = cp.tile([HD, CQ], fp16, tag="wo")
            bg_s = cp.tile([HD, 1], f32, tag="bg")
            bo_s = cp.tile([1, CQ], fp16, tag="bo")
            ones_r = cp.tile([1, 128], fp16, tag="onr")
            sel_s = cp.tile([128, H * 128], fp16, tag="sele")
            id_s = cp.tile([128, 128], fp16, tag="id")
            tri_s = cp.tile([128, 2 * H * N], fp16, tag="tri")
            mk_all = cp.tile([128, rows, 2], f32, tag="mkall")
            for t, d in ((m_s, mcat), (wv_s, wvT), (wg_s, wgT), (wo_s, woT),
                         (bg_s, bgc), (bo_s, bor), (ones_r, onesr),
                         (sel_s, sele), (id_s, id16)):
                nc.sync.dma_start(t[:], d[:])
            for i in range(2 * H):
                nc.sync.dma_start(tri_s[:, i * N:(i + 1) * N], triT[i])
            nc.sync.dma_start(mk_all[:], maskc.rearrange("r p t -> p r t"))
            # warmup: transpose instructions (S3_LW) can carry only one sync
            # wait; let PE observe the identity's DMA before the row loop.
            psW = ps.tile([128, 128], fp16, tag="psX")
            nc.tensor.transpose(psW[:], id_s[:], id_s[:])

            for n in range(rows):
                # ---- fp32->fp16 cast loads (gpsimd SWDGE casts) + PE
                # transpose (fp32 xbar DMA transpose unsupported) ----
                xqn = sbp.tile([128, 2, 128], fp16, tag="xqn")  # [tok%128, tok//128, c]
                xkn = sbp.tile([128, 2, 128], fp16, tag="xkn")
                nc.gpsimd.dma_start(xqn[:], qx[n].rearrange("(t p) c -> p t c", p=128))
                nc.gpsimd.dma_start(xkn[:], kvx[n].rearrange("(t p) c -> p t c", p=128))
                psX = ps.tile([128, 2 * N], fp16, tag="psX")
                for t in range(2):
                    nc.tensor.transpose(psX[:, t * 128:t * 128 + 128],
                                        xqn[:, t, :], id_s[:])
                    nc.tensor.transpose(psX[:, N + t * 128:N + t * 128 + 128],
                                        xkn[:, t, :], id_s[:])
                xqT = sbp.tile([CQ, N], fp16, tag="xqT")
                xkT = sbp.tile([CQ, N], fp16, tag="xkT")
                nc.vector.tensor_copy(xqT[:], psX[:, 0:N])
                nc.vector.tensor_copy(xkT[:], psX[:, N:2 * N])

                # ---- scores stage 1: u_h = (wk_h.T wq_h * scale * 256).T @ xkT
                psU = ps.tile([128, H * N], f32, tag="psX")  # shares psX slot
                for h in range(H):
                    nc.tensor.matmul(psU[:, h * N:(h + 1) * N],
                                     m_s[:, h * CQ:(h + 1) * CQ], xkT[:],
                                     start=True, stop=True)
                u_sb = sb.tile([128, H * N], fp16, tag="u")
                nc.vector.tensor_copy(u_sb[:, 0:2 * N], psU[:, 0:2 * N])
                nc.scalar.activation(u_sb[:, 2 * N:4 * N], psU[:, 2 * N:4 * N], AF.Copy)

                # ---- g lin + v projections ----
                psB = ps.tile([128, 2 * N], f32, tag="psB")  # gT | v(2 tok-tiles)
                nc.tensor.matmul(psB[:, 0:N], wg_s[:], xqT[:], start=True, stop=True)
                nc.tensor.matmul(psB[:, N:N + 128], xkT[:, 0:128], wv_s[:],
                                 start=True, stop=True)
                nc.tensor.matmul(psB[:, N + 128:2 * N], xkT[:, 128:N], wv_s[:],
                                 start=True, stop=True)
                # gating via tanh (same ACT table set as exp):
                # sigmoid(x) = 0.5*(1+tanh(x/2)); the 0.5 is folded into sele=2
                tT = sb.tile([128, N], f32, tag="tT")
                nc.scalar.activation(tT[:], psB[:, 0:N], AF.Tanh,
                                     scale=0.5, bias=bg_s[:, 0:1])
                gT = sb.tile([128, N], f32, tag="gT")
                nc.vector.tensor_scalar_add(gT[:], tT[:], 1.0)
                v_sb = sb.tile([128, N], fp16, tag="v")
                nc.vector.tensor_copy(v_sb[:], psB[:, N:2 * N])

                # ---- scores + triangle + exp((.)/256 + mask) ----
                pT = []
                for kt in range(2):
                    psS = ps2.tile([128, H * N], f32, tag="psS")  # 2 banks
                    for h in range(H):
                        nc.tensor.matmul(psS[:, h * N:(h + 1) * N], id_s[:],
                                         tri_s[:, (2 * h + kt) * N:(2 * h + kt + 1) * N],
                                         start=(h % 2 == 0), stop=False)
                    for h in range(H):
                        nc.tensor.matmul(
                            psS[:, h * N:(h + 1) * N],
                            u_sb[:, h * N + kt * 128:h * N + kt * 128 + 128],
                            xqT[:],
                            start=False, stop=(h % 2 == 1))
                    pTk = sb.tile([128, H * N], fp16, tag=f"pT{kt}")
                    nc.scalar.activation(pTk[:], psS[:], AF.Exp,
                                         scale=float(1.0 / 256.0),
                                         bias=mk_all[:, n, kt:kt + 1])
                    pT.append(pTk)

                # ---- AV (oT, col-tiled) + broadcast denominators ----
                psO = ps.tile([128, 2 * N], f32, tag="psO")
                for kt in range(2):
                    for h in range(H):
                        nc.tensor.matmul(
                            psO[32 * h:32 * h + 32, 0:N],
                            v_sb[:, kt * 128 + 32 * h:kt * 128 + 32 * h + 32],
                            pT[kt][:, h * N:(h + 1) * N],
                            start=(kt == 0), stop=(kt == 1),
                            tile_position=(0, 32 * h), skip_group_check=True)
                for kt in range(2):
                    for h in range(H):
                        nc.tensor.matmul(psO[:, N:2 * N],
                                         sel_s[:, 128 * h:128 * (h + 1)],
                                         pT[kt][:, h * N:(h + 1) * N],
                                         start=(kt == 0 and h == 0),
                                         stop=(kt == 1 and h == H - 1),
                                         skip_group_check=True)
                rb_s = sb.tile([128, N], f32, tag="rb")
                nc.vector.reciprocal_approx_fast(rb_s[:], psO[:, N:2 * N])

                # ---- gate * normalize, final projection (natural out) ----
                og = sb.tile([128, N], f32, tag="og")
                nc.vector.tensor_mul(og[:], psO[:, 0:N], gT[:])
                og2 = sb.tile([128, N], fp16, tag="og2")
                nc.vector.tensor_mul(og2[:], og[:], rb_s[:])
                psF = ps.tile([128, N], f32, tag="psB")
                for qt in range(2):
                    nc.tensor.matmul(psF[:, qt * 128:(qt + 1) * 128],
                                     og2[:, qt * 128:(qt + 1) * 128], wo_s[:],
                                     start=(qt == 0), stop=False)
                    nc.tensor.matmul(psF[:, qt * 128:(qt + 1) * 128],
                                     ones_r[:], bo_s[:], start=False,
                                     stop=(qt == 1))
                o_sb = sb.tile([128, N], f32, tag="osb")
                nc.vector.tensor_copy(o_sb[:], psF[:])
                for qt in range(2):
                    nc.sync.dma_start(out[n, qt * 128:(qt + 1) * 128, :],
                                      o_sb[:, qt * 128:(qt + 1) * 128])
    nc.compile()
    return nc


_PROG_CACHE = {}


def host_prep(q_x, kv_x, mask_bias, triangle_bias, wq, wk, wv, wg, bg, wo, bo):
    """Returns (qx [N,N,C], kvx, maskc [N,128,2], shared-constants dict)."""
    scale = np.float64(1.0 / np.float64(np.sqrt(np.float32(CH), dtype=np.float32)))
    q_x = np.ascontiguousarray(np.asarray(q_x, np.float32).reshape(N, N, CQ))
    kv_x = np.ascontiguousarray(np.asarray(kv_x, np.float32).reshape(N, N, CQ))

    wqf = np.asarray(wq, np.float64).reshape(H, CH, CQ)
    wkf = np.asarray(wk, np.float64).reshape(H, CH, CQ)
    # M_h = wk_h.T @ wq_h * scale * 256 (x256 dodges fp16 subnormals;
    # exp's scale=1/256 compensates), mcat [c, h*CQ + c']
    mcat = np.concatenate(
        [(wkf[h].T @ wqf[h] * (scale * 256.0)) for h in range(H)],
        axis=1).astype(np.float16)
    mcat = np.ascontiguousarray(mcat)
    wvT = np.ascontiguousarray(np.asarray(wv).reshape(HD, CQ).T.astype(np.float16))
    wgT = np.ascontiguousarray(np.asarray(wg).reshape(HD, CQ).T.astype(np.float16))
    woT = np.ascontiguousarray(np.asarray(wo).T.astype(np.float16))  # [e, c]
    bgc = np.ascontiguousarray(np.asarray(bg, np.float32).reshape(HD, 1) * 0.5)
    bor = np.ascontiguousarray(np.asarray(bo).reshape(1, CQ).astype(np.float16))
    onesr = np.ones((1, 128), np.float16)
    sele = np.zeros((128, H * 128), np.float16)
    for h in range(H):
        sele[:, 128 * h + 32 * h:128 * h + 32 * h + 32] = 2.0
    id16 = np.eye(128, dtype=np.float16)
    # mask: [n, k] -> [n, k_in_tile, kt]
    m = np.asarray(mask_bias, np.float32).reshape(N, N)
    maskc = np.ascontiguousarray(m.reshape(N, 2, 128).transpose(0, 2, 1))
    # triangle: [h, q, k] -> [(h,kt), k_in_tile, q], x256
    t = np.asarray(triangle_bias, np.float64).reshape(H, N, N) * 256.0
    triT = np.ascontiguousarray(
        t.transpose(0, 2, 1).reshape(H, 2, 128, N).reshape(2 * H, 128, N)
        .astype(np.float16))
    shared = dict(mcat=mcat, wvT=wvT, wgT=wgT, woT=woT, bgc=bgc, onesr=onesr,
                  bor=bor, sele=sele, id16=id16, triT=triT)
    return q_x, kv_x, maskc, shared


def make_in_maps(q_x, kv_x, mask_bias, triangle_bias, wq, wk, wv, wg, bg, wo, bo):
    qx, kvx, maskc, shared = host_prep(q_x, kv_x, mask_bias, triangle_bias,
                                       wq, wk, wv, wg, bg, wo, bo)
    in_maps = []
    for i in range(NCORES):
        sl = slice(i * ROWS, (i + 1) * ROWS)
        in_maps.append(dict(qx=np.ascontiguousarray(qx[sl]),
                            kvx=np.ascontiguousarray(kvx[sl]),
                            maskc=np.ascontiguousarray(maskc[sl]), **shared))
    return in_maps


def get_program():
    if ROWS not in _PROG_CACHE:
        _PROG_CACHE[ROWS] = build_program(ROWS)
    return _PROG_CACHE[ROWS]


def kernel(q_x, kv_x, mask_bias, triangle_bias, wq, wk, wv, wg, bg, wo, bo):
    from concourse.bass_utils import run_bass_kernel_spmd

    in_maps = make_in_maps(q_x, kv_x, mask_bias, triangle_bias,
                           wq, wk, wv, wg, bg, wo, bo)
    nc = get_program()
    res = run_bass_kernel_spmd(nc, in_maps, list(range(NCORES)))
    outs = [np.asarray(res.results[i]["out"]) for i in range(NCORES)]
    return np.concatenate(outs, axis=0)[None].astype(np.float32)


# revision 23
# speedup vs baseline: 1.1158x; 1.1158x over previous
"""Trainium2 Bass kernel for triangle (AlphaFold-style) gated attention over pair rows.

Problem: B=1, N=256 rows; per row n: attention over 256 positions,
H=4 heads x CH=32, C=128 channels, additive mask bias (per row, per key),
triangle bias (per head, q, k; shared across rows), sigmoid gating,
output projection. Rows sharded across 8 NeuronCores (32 rows/core), SPMD.

Per-core dataflow (transposed so the softmax key-reduction lands on the
PE partition axis; all matmul operands fp16 = single-pass PE):
  - load X natural (fp32->fp16 cast DMA on gpsimd), PE-transpose to
    xqT/xkT [c=128, tok=256] fp16
  - scores via host-precomputed M_h = wk_h.T wq_h * scale * 256 (fp16;
    the x256 keeps M out of fp16 subnormals; exp compensates with
    scale=1/256): u_h = M_h.T @ xkT, sT_h = u_slice.T @ xqT (all K=128,
    base partition 0 - row-tiled K=32 matmuls crash this HW)
  - triangle bias (x256, fp16) added by identity-matmul accumulation
  - p = exp((sT+tri)/256 + mask) via one ACT op per k-tile [128,1024],
    mask is the per-partition bias; no max-subtraction needed
    (|s|+|tri| bounded, exp(-1e9)=0 exactly like the reference mask)
  - oT[hd,q] = sum_kt v_h.T @ p_h (col-tiled, fp16); denominators
    broadcast directly to [128,256] by block-expander matmuls (2.0 in
    head blocks; the 2.0 folds the tanh-form sigmoid's 0.5), then one
    reciprocal_approx_fast (~18 bits)
  - gating via tanh (same ACT table set as exp; sigmoid would force a
    ~2.7us table reload per row): g = 1 + tanh(lin/2 + bg/2)
  - out[q,c] = ((oT * g * rb) @ wo.T + 1 x bo) natural layout
"""
import numpy as np

B, N, CQ, H, CH = 1, 256, 128, 4, 32
NCORES = 8
ROWS = N // NCORES  # 32
HD = H * CH  # 128


def build_program(rows):
    import concourse.bass as bass
    import concourse.bacc as bacc
    import concourse.mybir as mybir
    from concourse import tile

    f32 = mybir.dt.float32
    fp16 = mybir.dt.float16
    AF = mybir.ActivationFunctionType
    nc = bacc.Bacc("TRN2", target_bir_lowering=False, debug=False)

    qx = nc.declare_dram_parameter("qx", [rows, N, CQ], f32, isOutput=False)
    kvx = nc.declare_dram_parameter("kvx", [rows, N, CQ], f32, isOutput=False)
    maskc = nc.declare_dram_parameter("maskc", [rows, 128, 2], f32, isOutput=False)
    triT = nc.declare_dram_parameter("triT", [2 * H, 128, N], fp16, isOutput=False)
    mcat = nc.declare_dram_parameter("mcat", [CQ, H * CQ], fp16, isOutput=False)
    wvT = nc.declare_dram_parameter("wvT", [CQ, HD], fp16, isOutput=False)
    wgT = nc.declare_dram_parameter("wgT", [CQ, HD], fp16, isOutput=False)
    woT = nc.declare_dram_parameter("woT", [HD, CQ], fp16, isOutput=False)
    bgc = nc.declare_dram_parameter("bgc", [HD, 1], f32, isOutput=False)
    bor = nc.declare_dram_parameter("bor", [1, CQ], fp16, isOutput=False)
    onesr = nc.declare_dram_parameter("onesr", [1, 128], fp16, isOutput=False)
    sele = nc.declare_dram_parameter("sele", [128, H * 128], fp16, isOutput=False)
    id16 = nc.declare_dram_parameter("id16", [128, 128], fp16, isOutput=False)
    out = nc.declare_dram_parameter("out", [rows, N, CQ], f32, isOutput=True)

    with tile.TileContext(nc) as tc:
        with (
            nc.allow_low_precision(reason="fp16 matmul operands and "
                                   "reciprocal_approx_fast by design"),
            tc.tile_pool(name="const", bufs=1) as cp,
            tc.tile_pool(name="sb", bufs=2) as sb,
            tc.tile_pool(name="sbp", bufs=3) as sbp,
            tc.tile_pool(name="ps", bufs=1, space=bass.MemorySpace.PSUM) as ps,
        ):
            # ---- constants ----
            m_s = cp.tile([CQ, H * CQ], fp16, tag="mcat")
            wv_s = cp.tile([CQ, HD], fp16, tag="wv")
            wg_s = cp.tile([CQ, HD], fp16, tag="wg")
            wo_s = cp.tile([HD, CQ], fp16, tag="wo")
            bg_s = cp.tile([HD, 1], f32, tag="bg")
            bo_s = cp.tile([1, CQ], fp16, tag="bo")
            ones_r = cp.tile([1, 128], fp16, tag="onr")
            sel_s = cp.tile([128, H * 128], fp16, tag="sele")
            id_s = cp.tile([128, 128], fp16, tag="id")
            tri_s = cp.tile([128, 2 * H * N], fp16, tag="tri")
            mk_all = cp.tile([128, rows, 2], f32, tag="mkall")
            for t, d in ((m_s, mcat), (wv_s, wvT), (wg_s, wgT), (wo_s, woT),
                         (bg_s, bgc), (bo_s, bor), (ones_r, onesr),
                         (sel_s, sele), (id_s, id16)):
                nc.sync.dma_start(t[:], d[:])
            for i in range(2 * H):
                nc.sync.dma_start(tri_s[:, i * N:(i + 1) * N], triT[i])
            nc.sync.dma_start(mk_all[:], maskc.rearrange("r p t -> p r t"))
            # warmup: transpose instructions (S3_LW) can carry only one sync
            # wait; let PE observe the identity's DMA before the row loop.
            psW = ps.tile([128, 128], fp16, tag="psX")
            nc.tensor.transpose(psW[:], id_s[:], id_s[:])

            for n in range(rows):
                # ---- fp32->fp16 cast loads (gpsimd SWDGE casts) + PE
                # transpose (fp32 xbar DMA transpose unsupported) ----
                xqn = sbp.tile([128, 2, 128], fp16, tag="xqn")  # [tok%128, tok//128, c]
                xkn = sbp.tile([128, 2, 128], fp16, tag="xkn")
                nc.gpsimd.dma_start(xqn[:], qx[n].rearrange("(t p) c -> p t c", p=128))
                nc.gpsimd.dma_start(xkn[:], kvx[n].rearrange("(t p) c -> p t c", p=128))
                psX = ps.tile([128, 2 * N], fp16, tag="psX")
                for t in range(2):
                    nc.tensor.transpose(psX[:, t * 128:t * 128 + 128],
                                        xqn[:, t, :], id_s[:])
                    nc.tensor.transpose(psX[:, N + t * 128:N + t * 128 + 128],
                                        xkn[:, t, :], id_s[:])
                xqT = sbp.tile([CQ, N], fp16, tag="xqT")
                xkT = sbp.tile([CQ, N], fp16, tag="xkT")
                nc.vector.tensor_copy(xqT[:], psX[:, 0:N])
                nc.vector.tensor_copy(xkT[:], psX[:, N:2 * N])

                # ---- scores stage 1: u_h = (wk_h.T wq_h * scale * 256).T @ xkT
                psU = ps.tile([128, H * N], f32, tag="psX")  # shares psX slot
                for h in range(H):
                    nc.tensor.matmul(psU[:, h * N:(h + 1) * N],
                                     m_s[:, h * CQ:(h + 1) * CQ], xkT[:],
                                     start=True, stop=True)
                u_sb = sb.tile([128, H * N], fp16, tag="u")
                nc.vector.tensor_copy(u_sb[:, 0:2 * N], psU[:, 0:2 * N])
                nc.scalar.activation(u_sb[:, 2 * N:4 * N], psU[:, 2 * N:4 * N], AF.Copy)

                # ---- g lin + v projections ----
                psB = ps.tile([128, 2 * N], f32, tag="psB")  # gT | v(2 tok-tiles)
                nc.tensor.matmul(psB[:, 0:N], wg_s[:], xqT[:], start=True, stop=True)
                nc.tensor.matmul(psB[:, N:N + 128], xkT[:, 0:128], wv_s[:],
                                 start=True, stop=True)
                nc.tensor.matmul(psB[:, N + 128:2 * N], xkT[:, 128:N], wv_s[:],
                                 start=True, stop=True)
                # gating via tanh (same ACT table set as exp):
                # sigmoid(x) = 0.5*(1+tanh(x/2)); the 0.5 is folded into sele=2
                tT = sb.tile([128, N], f32, tag="tT")
                nc.scalar.activation(tT[:], psB[:, 0:N], AF.Tanh,
                                     scale=0.5, bias=bg_s[:, 0:1])
                gT = sb.tile([128, N], f32, tag="gT")
                nc.vector.tensor_scalar_add(gT[:], tT[:], 1.0)
                v_sb = sb.tile([128, N], fp16, tag="v")
                nc.vector.tensor_copy(v_sb[:], psB[:, N:2 * N])

                # ---- scores + triangle + exp((.)/256 + mask) ----
                pT = []
                for kt in range(2):
                    psS = ps.tile([128, H * N], f32, tag="psS")  # 2 banks
                    for h in range(H):
                        nc.tensor.matmul(psS[:, h * N:(h + 1) * N], id_s[:],
                                         tri_s[:, (2 * h + kt) * N:(2 * h + kt + 1) * N],
                                         start=(h % 2 == 0), stop=False)
                    for h in range(H):
                        nc.tensor.matmul(
                            psS[:, h * N:(h + 1) * N],
                            u_sb[:, h * N + kt * 128:h * N + kt * 128 + 128],
                            xqT[:],
                            start=False, stop=(h % 2 == 1))
                    pTk = sb.tile([128, H * N], fp16, tag=f"pT{kt}")
                    nc.scalar.activation(pTk[:], psS[:], AF.Exp,
                                         scale=float(1.0 / 256.0),
                                         bias=mk_all[:, n, kt:kt + 1])
                    pT.append(pTk)

                # ---- AV (oT, col-tiled) + broadcast denominators ----
                psO = ps.tile([128, N], f32, tag="psO")
                for kt in range(2):
                    for h in range(H):
                        nc.tensor.matmul(
                            psO[32 * h:32 * h + 32, 0:N],
                            v_sb[:, kt * 128 + 32 * h:kt * 128 + 32 * h + 32],
                            pT[kt][:, h * N:(h + 1) * N],
                            start=(kt == 0), stop=(kt == 1),
                            tile_position=(0, 32 * h), skip_group_check=True)
                psD = ps.tile([128, N], f32, tag="psD")
                for kt in range(2):
                    for h in range(H):
                        nc.tensor.matmul(psD[:], sel_s[:, 128 * h:128 * (h + 1)],
                                         pT[kt][:, h * N:(h + 1) * N],
                                         start=(kt == 0 and h == 0),
                                         stop=(kt == 1 and h == H - 1))
                rb_s = sb.tile([128, N], f32, tag="rb")
                nc.vector.reciprocal_approx_fast(rb_s[:], psD[:])

                # ---- gate * normalize, final projection (natural out) ----
                og = sb.tile([128, N], f32, tag="og")
                nc.vector.tensor_mul(og[:], psO[:, 0:N], gT[:])
                og2 = sb.tile([128, N], fp16, tag="og2")
                nc.vector.tensor_mul(og2[:], og[:], rb_s[:])
                psF = ps.tile([128, N], f32, tag="psF")
                for qt in range(2):
                    nc.tensor.matmul(psF[:, qt * 128:(qt + 1) * 128],
                                     og2[:, qt * 128:(qt + 1) * 128], wo_s[:],
                                     start=(qt == 0), stop=False)
                    nc.tensor.matmul(psF[:, qt * 128:(qt + 1) * 128],
                                     ones_r[:], bo_s[:], start=False,
                                     stop=(qt == 1))
                o_sb = sb.tile([128, N], f32, tag="osb")
                nc.vector.tensor_copy(o_sb[:], psF[:])
                for qt in range(2):
                    nc.sync.dma_start(out[n, qt * 128:(qt + 1) * 128, :],
                                      o_sb[:, qt * 128:(qt + 1) * 128])
    nc.compile()
    return nc


_PROG_CACHE = {}


def host_prep(q_x, kv_x, mask_bias, triangle_bias, wq, wk, wv, wg, bg, wo, bo):
    """Returns (qx [N,N,C], kvx, maskc [N,128,2], shared-constants dict)."""
    scale = np.float64(1.0 / np.float64(np.sqrt(np.float32(CH), dtype=np.float32)))
    q_x = np.ascontiguousarray(np.asarray(q_x, np.float32).reshape(N, N, CQ))
    kv_x = np.ascontiguousarray(np.asarray(kv_x, np.float32).reshape(N, N, CQ))

    wqf = np.asarray(wq, np.float64).reshape(H, CH, CQ)
    wkf = np.asarray(wk, np.float64).reshape(H, CH, CQ)
    # M_h = wk_h.T @ wq_h * scale * 256 (x256 dodges fp16 subnormals;
    # exp's scale=1/256 compensates), mcat [c, h*CQ + c']
    mcat = np.concatenate(
        [(wkf[h].T @ wqf[h] * (scale * 256.0)) for h in range(H)],
        axis=1).astype(np.float16)
    mcat = np.ascontiguousarray(mcat)
    wvT = np.ascontiguousarray(np.asarray(wv).reshape(HD, CQ).T.astype(np.float16))
    wgT = np.ascontiguousarray(np.asarray(wg).reshape(HD, CQ).T.astype(np.float16))
    woT = np.ascontiguousarray(np.asarray(wo).T.astype(np.float16))  # [e, c]
    bgc = np.ascontiguousarray(np.asarray(bg, np.float32).reshape(HD, 1) * 0.5)
    bor = np.ascontiguousarray(np.asarray(bo).reshape(1, CQ).astype(np.float16))
    onesr = np.ones((1, 128), np.float16)
    sele = np.zeros((128, H * 128), np.float16)
    for h in range(H):
        sele[:, 128 * h + 32 * h:128 * h + 32 * h + 32] = 2.0
    id16 = np.eye(128, dtype=np.float16)
    # mask: [n, k] -> [n, k_in_tile, kt]
    m = np.asarray(mask_bias, np.float32).reshape(N, N)
    maskc = np.ascontiguousarray(m.reshape(N, 2, 128).transpose(0, 2, 1))
    # triangle: [h, q, k] -> [(h,kt), k_in_tile, q], x256
    t = np.asarray(triangle_bias, np.float64).reshape(H, N, N) * 256.0
    triT = np.ascontiguousarray(
        t.transpose(0, 2, 1).reshape(H, 2, 128, N).reshape(2 * H, 128, N)
        .astype(np.float16))
    shared = dict(mcat=mcat, wvT=wvT, wgT=wgT, woT=woT, bgc=bgc, onesr=onesr,
                  bor=bor, sele=sele, id16=id16, triT=triT)
    return q_x, kv_x, maskc, shared


def make_in_maps(q_x, kv_x, mask_bias, triangle_bias, wq, wk, wv, wg, bg, wo, bo):
    qx, kvx, maskc, shared = host_prep(q_x, kv_x, mask_bias, triangle_bias,
                                       wq, wk, wv, wg, bg, wo, bo)
    in_maps = []
    for i in range(NCORES):
        sl = slice(i * ROWS, (i + 1) * ROWS)
        in_maps.append(dict(qx=np.ascontiguousarray(qx[sl]),
                            kvx=np.ascontiguousarray(kvx[sl]),
                            maskc=np.ascontiguousarray(maskc[sl]), **shared))
    return in_maps


def get_program():
    if ROWS not in _PROG_CACHE:
        _PROG_CACHE[ROWS] = build_program(ROWS)
    return _PROG_CACHE[ROWS]


def kernel(q_x, kv_x, mask_bias, triangle_bias, wq, wk, wv, wg, bg, wo, bo):
    from concourse.bass_utils import run_bass_kernel_spmd

    in_maps = make_in_maps(q_x, kv_x, mask_bias, triangle_bias,
                           wq, wk, wv, wg, bg, wo, bo)
    nc = get_program()
    res = run_bass_kernel_spmd(nc, in_maps, list(range(NCORES)))
    outs = [np.asarray(res.results[i]["out"]) for i in range(NCORES)]
    return np.concatenate(outs, axis=0)[None].astype(np.float32)


# revision 24
# speedup vs baseline: 1.1814x; 1.0588x over previous
"""Trainium2 Bass kernel for triangle (AlphaFold-style) gated attention over pair rows.

Problem: B=1, N=256 rows; per row n: attention over 256 positions,
H=4 heads x CH=32, C=128 channels, additive mask bias (per row, per key),
triangle bias (per head, q, k; shared across rows), sigmoid gating,
output projection. Rows sharded across 8 NeuronCores (32 rows/core), SPMD.

Per-core dataflow (transposed so the softmax key-reduction lands on the
PE partition axis; all matmul operands fp16 = single-pass PE):
  - load X natural (fp32->fp16 cast DMA on gpsimd), PE-transpose to
    xqT/xkT [c=128, tok=256] fp16
  - scores via host-precomputed M_h = wk_h.T wq_h * scale * 256 (fp16;
    the x256 keeps M out of fp16 subnormals; exp compensates with
    scale=1/256): u_h = M_h.T @ xkT, sT_h = u_slice.T @ xqT (all K=128,
    base partition 0 - row-tiled K=32 matmuls crash this HW)
  - triangle bias (x256, fp16) added by identity-matmul accumulation
  - p = exp((sT+tri)/256 + mask) via one ACT op per k-tile [128,1024],
    mask is the per-partition bias; no max-subtraction needed
    (|s|+|tri| bounded, exp(-1e9)=0 exactly like the reference mask)
  - oT[hd,q] = sum_kt v_h.T @ p_h (col-tiled, fp16); denominators
    broadcast directly to [128,256] by block-expander matmuls (2.0 in
    head blocks; the 2.0 folds the tanh-form sigmoid's 0.5), then one
    reciprocal_approx_fast (~18 bits)
  - gating via tanh (same ACT table set as exp; sigmoid would force a
    ~2.7us table reload per row): g = 1 + tanh(lin/2 + bg/2)
  - out[q,c] = ((oT * g * rb) @ wo.T + 1 x bo) natural layout
"""
import numpy as np

B, N, CQ, H, CH = 1, 256, 128, 4, 32
NCORES = 8
ROWS = N // NCORES  # 32
HD = H * CH  # 128


def build_program(rows):
    import concourse.bass as bass
    import concourse.bacc as bacc
    import concourse.mybir as mybir
    from concourse import tile

    f32 = mybir.dt.float32
    fp16 = mybir.dt.float16
    AF = mybir.ActivationFunctionType
    nc = bacc.Bacc("TRN2", target_bir_lowering=False, debug=False)

    qx = nc.declare_dram_parameter("qx", [rows, N, CQ], f32, isOutput=False)
    kvx = nc.declare_dram_parameter("kvx", [rows, N, CQ], f32, isOutput=False)
    maskc = nc.declare_dram_parameter("maskc", [rows, 128, 2], f32, isOutput=False)
    triT = nc.declare_dram_parameter("triT", [2 * H, 128, N], fp16, isOutput=False)
    mcat = nc.declare_dram_parameter("mcat", [CQ, H * CQ], fp16, isOutput=False)
    wvT = nc.declare_dram_parameter("wvT", [CQ, HD], fp16, isOutput=False)
    wgT = nc.declare_dram_parameter("wgT", [CQ, HD], fp16, isOutput=False)
    woT = nc.declare_dram_parameter("woT", [HD, CQ], fp16, isOutput=False)
    bgc = nc.declare_dram_parameter("bgc", [HD, 1], f32, isOutput=False)
    bor = nc.declare_dram_parameter("bor", [1, CQ], fp16, isOutput=False)
    onesr = nc.declare_dram_parameter("onesr", [1, 128], fp16, isOutput=False)
    sele = nc.declare_dram_parameter("sele", [128, 32], fp16, isOutput=False)
    id16 = nc.declare_dram_parameter("id16", [128, 128], fp16, isOutput=False)
    out = nc.declare_dram_parameter("out", [rows, N, CQ], f32, isOutput=True)

    with tile.TileContext(nc) as tc:
        with (
            nc.allow_low_precision(reason="fp16 matmul operands and "
                                   "reciprocal_approx_fast by design"),
            tc.tile_pool(name="const", bufs=1) as cp,
            tc.tile_pool(name="sb", bufs=2) as sb,
            tc.tile_pool(name="sbp", bufs=3) as sbp,
            tc.tile_pool(name="ps", bufs=1, space=bass.MemorySpace.PSUM) as ps,
        ):
            # ---- constants ----
            m_s = cp.tile([CQ, H * CQ], fp16, tag="mcat")
            wv_s = cp.tile([CQ, HD], fp16, tag="wv")
            wg_s = cp.tile([CQ, HD], fp16, tag="wg")
            wo_s = cp.tile([HD, CQ], fp16, tag="wo")
            bg_s = cp.tile([HD, 1], f32, tag="bg")
            bo_s = cp.tile([1, CQ], fp16, tag="bo")
            ones_r = cp.tile([1, 128], fp16, tag="onr")
            sel_s = cp.tile([128, 32], fp16, tag="sele")
            id_s = cp.tile([128, 128], fp16, tag="id")
            tri_s = cp.tile([128, 2 * H * N], fp16, tag="tri")
            mk_all = cp.tile([128, rows, 2], f32, tag="mkall")
            for t, d in ((m_s, mcat), (wv_s, wvT), (wg_s, wgT), (wo_s, woT),
                         (bg_s, bgc), (bo_s, bor), (ones_r, onesr),
                         (sel_s, sele), (id_s, id16)):
                nc.sync.dma_start(t[:], d[:])
            for i in range(2 * H):
                nc.sync.dma_start(tri_s[:, i * N:(i + 1) * N], triT[i])
            nc.sync.dma_start(mk_all[:], maskc.rearrange("r p t -> p r t"))
            # warmup: transpose instructions (S3_LW) can carry only one sync
            # wait; let PE observe the identity's DMA before the row loop.
            psW = ps.tile([128, 128], fp16, tag="psX")
            nc.tensor.transpose(psW[:], id_s[:], id_s[:])

            for n in range(rows):
                # ---- fp32->fp16 cast loads (gpsimd SWDGE casts) + PE
                # transpose (fp32 xbar DMA transpose unsupported) ----
                xqn = sbp.tile([128, 2, 128], fp16, tag="xqn")  # [tok%128, tok//128, c]
                xkn = sbp.tile([128, 2, 128], fp16, tag="xkn")
                nc.gpsimd.dma_start(xqn[:], qx[n].rearrange("(t p) c -> p t c", p=128))
                nc.gpsimd.dma_start(xkn[:], kvx[n].rearrange("(t p) c -> p t c", p=128))
                psX = ps.tile([128, 2 * N], fp16, tag="psX")
                for t in range(2):
                    nc.tensor.transpose(psX[:, t * 128:t * 128 + 128],
                                        xqn[:, t, :], id_s[:])
                    nc.tensor.transpose(psX[:, N + t * 128:N + t * 128 + 128],
                                        xkn[:, t, :], id_s[:])
                xqT = sbp.tile([CQ, N], fp16, tag="xqT")
                xkT = sbp.tile([CQ, N], fp16, tag="xkT")
                nc.vector.tensor_copy(xqT[:], psX[:, 0:N])
                nc.vector.tensor_copy(xkT[:], psX[:, N:2 * N])

                # ---- scores stage 1: u_h = (wk_h.T wq_h * scale * 256).T @ xkT
                psU = ps.tile([128, H * N], f32, tag="psX")  # shares psX slot
                for h in range(H):
                    nc.tensor.matmul(psU[:, h * N:(h + 1) * N],
                                     m_s[:, h * CQ:(h + 1) * CQ], xkT[:],
                                     start=True, stop=True)
                u_sb = sb.tile([128, H * N], fp16, tag="u")
                nc.vector.tensor_copy(u_sb[:, 0:2 * N], psU[:, 0:2 * N])
                nc.scalar.activation(u_sb[:, 2 * N:4 * N], psU[:, 2 * N:4 * N], AF.Copy)

                # ---- g lin + v projections ----
                psB = ps.tile([128, 2 * N], f32, tag="psB")  # gT | v(2 tok-tiles)
                nc.tensor.matmul(psB[:, 0:N], wg_s[:], xqT[:], start=True, stop=True)
                nc.tensor.matmul(psB[:, N:N + 128], xkT[:, 0:128], wv_s[:],
                                 start=True, stop=True)
                nc.tensor.matmul(psB[:, N + 128:2 * N], xkT[:, 128:N], wv_s[:],
                                 start=True, stop=True)
                # gating via tanh (same ACT table set as exp):
                # sigmoid(x) = 0.5*(1+tanh(x/2)); the 0.5 is folded into sele=2
                tT = sb.tile([128, N], f32, tag="tT")
                nc.scalar.activation(tT[:], psB[:, 0:N], AF.Tanh,
                                     scale=0.5, bias=bg_s[:, 0:1])
                gT = sb.tile([128, N], f32, tag="gT")
                nc.vector.tensor_scalar_add(gT[:], tT[:], 1.0)
                v_sb = sb.tile([128, N], fp16, tag="v")
                nc.vector.tensor_copy(v_sb[:], psB[:, N:2 * N])

                # ---- scores + triangle + exp((.)/256 + mask) ----
                pT = []
                for kt in range(2):
                    psS = ps.tile([128, H * N], f32, tag="psS")  # 2 banks
                    for h in range(H):
                        nc.tensor.matmul(psS[:, h * N:(h + 1) * N], id_s[:],
                                         tri_s[:, (2 * h + kt) * N:(2 * h + kt + 1) * N],
                                         start=(h % 2 == 0), stop=False)
                    for h in range(H):
                        nc.tensor.matmul(
                            psS[:, h * N:(h + 1) * N],
                            u_sb[:, h * N + kt * 128:h * N + kt * 128 + 128],
                            xqT[:],
                            start=False, stop=(h % 2 == 1))
                    pTk = sb.tile([128, H * N], fp16, tag=f"pT{kt}")
                    nc.scalar.activation(pTk[:], psS[:], AF.Exp,
                                         scale=float(1.0 / 256.0),
                                         bias=mk_all[:, n, kt:kt + 1])
                    pT.append(pTk)

                # ---- AV (oT, col-tiled) + broadcast denominators ----
                psO = ps.tile([128, N], f32, tag="psO")
                for kt in range(2):
                    for h in range(H):
                        nc.tensor.matmul(
                            psO[32 * h:32 * h + 32, 0:N],
                            v_sb[:, kt * 128 + 32 * h:kt * 128 + 32 * h + 32],
                            pT[kt][:, h * N:(h + 1) * N],
                            start=(kt == 0), stop=(kt == 1),
                            tile_position=(0, 32 * h), skip_group_check=True)
                psD = ps.tile([128, N], f32, tag="psD")
                for kt in range(2):
                    for h in range(H):
                        nc.tensor.matmul(psD[32 * h:32 * h + 32, :], sel_s[:],
                                         pT[kt][:, h * N:(h + 1) * N],
                                         start=(kt == 0), stop=(kt == 1),
                                         tile_position=(0, 32 * h),
                                         skip_group_check=True)
                rb_s = sb.tile([128, N], f32, tag="rb")
                nc.vector.reciprocal_approx_fast(rb_s[:], psD[:])

                # ---- gate * normalize, final projection (natural out) ----
                og = sb.tile([128, N], f32, tag="og")
                nc.vector.tensor_mul(og[:], psO[:, 0:N], gT[:])
                og2 = sb.tile([128, N], fp16, tag="og2")
                nc.vector.tensor_mul(og2[:], og[:], rb_s[:])
                psF = ps.tile([128, N], f32, tag="psF")
                for qt in range(2):
                    nc.tensor.matmul(psF[:, qt * 128:(qt + 1) * 128],
                                     og2[:, qt * 128:(qt + 1) * 128], wo_s[:],
                                     start=(qt == 0), stop=False)
                    nc.tensor.matmul(psF[:, qt * 128:(qt + 1) * 128],
                                     ones_r[:], bo_s[:], start=False,
                                     stop=(qt == 1))
                o_sb = sb.tile([128, N], f32, tag="osb")
                nc.vector.tensor_copy(o_sb[:], psF[:])
                for qt in range(2):
                    nc.sync.dma_start(out[n, qt * 128:(qt + 1) * 128, :],
                                      o_sb[:, qt * 128:(qt + 1) * 128])
    nc.compile()
    return nc


_PROG_CACHE = {}


def host_prep(q_x, kv_x, mask_bias, triangle_bias, wq, wk, wv, wg, bg, wo, bo):
    """Returns (qx [N,N,C], kvx, maskc [N,128,2], shared-constants dict)."""
    scale = np.float64(1.0 / np.float64(np.sqrt(np.float32(CH), dtype=np.float32)))
    q_x = np.ascontiguousarray(np.asarray(q_x, np.float32).reshape(N, N, CQ))
    kv_x = np.ascontiguousarray(np.asarray(kv_x, np.float32).reshape(N, N, CQ))

    wqf = np.asarray(wq, np.float64).reshape(H, CH, CQ)
    wkf = np.asarray(wk, np.float64).reshape(H, CH, CQ)
    # M_h = wk_h.T @ wq_h * scale * 256 (x256 dodges fp16 subnormals;
    # exp's scale=1/256 compensates), mcat [c, h*CQ + c']
    mcat = np.concatenate(
        [(wkf[h].T @ wqf[h] * (scale * 256.0)) for h in range(H)],
        axis=1).astype(np.float16)
    mcat = np.ascontiguousarray(mcat)
    wvT = np.ascontiguousarray(np.asarray(wv).reshape(HD, CQ).T.astype(np.float16))
    wgT = np.ascontiguousarray(np.asarray(wg).reshape(HD, CQ).T.astype(np.float16))
    woT = np.ascontiguousarray(np.asarray(wo).T.astype(np.float16))  # [e, c]
    bgc = np.ascontiguousarray(np.asarray(bg, np.float32).reshape(HD, 1) * 0.5)
    bor = np.ascontiguousarray(np.asarray(bo).reshape(1, CQ).astype(np.float16))
    onesr = np.ones((1, 128), np.float16)
    sele = np.full((128, 32), 2.0, np.float16)
    id16 = np.eye(128, dtype=np.float16)
    # mask: [n, k] -> [n, k_in_tile, kt]
    m = np.asarray(mask_bias, np.float32).reshape(N, N)
    maskc = np.ascontiguousarray(m.reshape(N, 2, 128).transpose(0, 2, 1))
    # triangle: [h, q, k] -> [(h,kt), k_in_tile, q], x256
    t = np.asarray(triangle_bias, np.float64).reshape(H, N, N) * 256.0
    triT = np.ascontiguousarray(
        t.transpose(0, 2, 1).reshape(H, 2, 128, N).reshape(2 * H, 128, N)
        .astype(np.float16))
    shared = dict(mcat=mcat, wvT=wvT, wgT=wgT, woT=woT, bgc=bgc, onesr=onesr,
                  bor=bor, sele=sele, id16=id16, triT=triT)
    return q_x, kv_x, maskc, shared


def make_in_maps(q_x, kv_x, mask_bias, triangle_bias, wq, wk, wv, wg, bg, wo, bo):
    qx, kvx, maskc, shared = host_prep(q_x, kv_x, mask_bias, triangle_bias,
                                       wq, wk, wv, wg, bg, wo, bo)
    in_maps = []
    for i in range(NCORES):
        sl = slice(i * ROWS, (i + 1) * ROWS)
        in_maps.append(dict(qx=np.ascontiguousarray(qx[sl]),
                            kvx=np.ascontiguousarray(kvx[sl]),
                            maskc=np.ascontiguousarray(maskc[sl]), **shared))
    return in_maps


def get_program():
    if ROWS not in _PROG_CACHE:
        _PROG_CACHE[ROWS] = build_program(ROWS)
    return _PROG_CACHE[ROWS]


def kernel(q_x, kv_x, mask_bias, triangle_bias, wq, wk, wv, wg, bg, wo, bo):
    from concourse.bass_utils import run_bass_kernel_spmd

    in_maps = make_in_maps(q_x, kv_x, mask_bias, triangle_bias,
                           wq, wk, wv, wg, bg, wo, bo)
    nc = get_program()
    res = run_bass_kernel_spmd(nc, in_maps, list(range(NCORES)))
    outs = [np.asarray(res.results[i]["out"]) for i in range(NCORES)]
    return np.concatenate(outs, axis=0)[None].astype(np.float32)
